# revision 21
# baseline (speedup 1.0000x reference)
"""Dilated-attention (SEG=512, DIL=2) fused kernel for TRN2, 8 NeuronCores.

v2: all matmul operands in bf16 (fp32 PSUM accumulate, fp32 LN statistics),
transposed-score attention, causal-aware tile sizes, N=512 moving dims.

Strategy: data-parallel over the 32 (batch, segment) attention blocks; each of
the 8 cores owns 4 blocks = 1024 active (even-position) tokens.  Odd token
positions contribute exactly zero to the output and are never computed.

Math layout (per core):
  - W columns are centered on host, so x @ W'.T is mean-free; only
    sum-of-squares is computed on device (fp32, via ones-matmuls on PE).
  - q~ = x @ Wq'.T evicted bf16 (gamma folded per-partition into the
    eviction); rstd_q/32 is applied EXACTLY in fp32 by a DVE multiply on the
    transposed score PSUM (free dim = q).
  - k~ evicted bf16 (gamma folded); rstd_k applied EXACTLY as the
    per-partition fp32 scale inside the ACT Exp eviction of ST[k, q].
  - softmax runs unnormalized: pts = Exp(rk*(rq*ST)) * maskT * rstd_v; the
    denominator l (per q) is divided out at the FINAL output eviction in
    fp32.  |scores| <= 32 so Exp never overflows.
  - v evicted bf16 token-major (gamma via row broadcast when != 1); rstd_v
    applied exactly (fp32 col) in the mask op.
  - AV and out-proj consume bf16; out-proj runs per token half, overlapping
    the second half of attention and shrinking the output tail.
  - beta != 0 falls back to extra bf16 post-passes producing fully
    normalized q/k/v (exact handling, slightly slower; untaken in practice).
"""

import numpy as np
import ml_dtypes

import concourse.bass as bass
import concourse.mybir as mybir
import concourse.tile as tile
from concourse import bacc
from concourse.bass_utils import run_bass_kernel_spmd

AFT = mybir.ActivationFunctionType
ALU = mybir.AluOpType
AX = mybir.AxisListType
F32 = mybir.dt.float32
F32R = mybir.dt.float32r
BF16 = mybir.dt.bfloat16
NPBF = ml_dtypes.bfloat16

B, N, D = 4, 4096, 1024
SEG, DIL = 512, 2
NSEG = N // SEG          # 8 segments per batch row
S = SEG // DIL           # 256 active tokens per segment
NCORES = 8
PAIRS = B * NSEG         # 32 (b, g) attention blocks
PPC = PAIRS // NCORES    # 4 blocks per core
TOK = PPC * S            # 1024 tokens per core
NE = D // 128            # 8 feature tiles
ND = D // 128            # 8 contraction tiles
NTT = TOK // 128         # 8 token tiles
LN_EPS = 1e-5

_CACHE: dict = {}


def _build_module(gamma_trivial: bool, beta_nonzero: bool, dbg: bool = False):
    nc = bacc.Bacc(
        "TRN2",
        target_bir_lowering=False,
        debug=False,
        enable_asserts=False,
        num_devices=NCORES,
    )

    def din(name, shape, dt=BF16):
        return nc.dram_tensor(name, shape, dt, kind="ExternalInput").ap()

    aps = dict(
        xt_d=din("xt", [2, 128, ND, 512]),        # x.T tiles per h-half
        wq_d=din("wq", [NE, 128, ND, 128]),       # stationary [d, e] tiles
        wk_d=din("wk", [NE, 128, ND, 128]),
        wv_d=din("wv", [128, ND, D]),             # moving [d, f] tiles
        wo_d=din("wo", [128, NE, D]),             # stationary [e, f] tiles
        bm_d=din("bm", [128, 256]),               # maskT[k, q] = (q >= k)
        onc_d=din("onc", [128, 1]),               # bf16 ones column
        or32_d=din("or32", [1, 128], F32),        # fp32 ones row (broadcasts)
        orb_d=din("orb", [1, 128]),               # bf16 ones row
        o11_d=din("o11", [1, 1], F32),            # fp32 [1,1] one (columnize)
        gc_d=din("gc", [128, NE], F32),           # gamma columns
        gr_d=din("gr", [1, D]),                   # gamma row (bf16)
        bc_d=din("bc", [128, NE], F32),           # beta columns
        br_d=din("br", [1, D]),                   # beta row (bf16)
        ot_d=nc.dram_tensor("ot", [128, NE, PPC, S], F32,
                            kind="ExternalOutput").ap(),
    )
    if dbg:
        for nm, shape, dt in (
                ("dqc", [128, NE, TOK], BF16), ("dkc", [128, NE, TOK], BF16),
                ("dvc", [128, NTT, D], BF16), ("drqb", [128, TOK], F32),
                ("drk", [128, 2 * PPC], F32), ("drv", [128, NTT], F32),
                ("dpts0", [128, PPC, 256], BF16),
                ("dpts1", [128, PPC, 128], BF16),
                ("drl", [2, 128, 512], F32),
                ("dyb", [2, 128, NE, 512], BF16)):
            aps[nm[1:] + "_dbg"] = nc.dram_tensor(
                nm, shape, dt, kind="ExternalOutput").ap()

    with tile.TileContext(nc) as tc:
        _body(tc, gamma_trivial, beta_nonzero, dbg=dbg, **aps)
    nc.compile()
    return nc


def _body(tc, gamma_trivial, beta_nonzero, xt_d, wq_d, wk_d, wv_d, wo_d,
          bm_d, onc_d, or32_d, orb_d, o11_d, gc_d, gr_d, bc_d, br_d,
          ot_d, dbg=False, qc_dbg=None, kc_dbg=None, vc_dbg=None,
          rqb_dbg=None, rk_dbg=None, rv_dbg=None, pts0_dbg=None,
          pts1_dbg=None, rl_dbg=None, yb_dbg=None):
    from contextlib import ExitStack

    nc = tc.nc
    with ExitStack() as ctx:
        ec = ctx.enter_context

        const_p = ec(tc.tile_pool(name="const", bufs=1))
        big_p = ec(tc.tile_pool(name="big", bufs=1))
        ws_p = ec(tc.tile_pool(name="ws", bufs=3))
        sq_p = ec(tc.tile_pool(name="sq", bufs=3))
        p01_p = ec(tc.tile_pool(name="p01", bufs=2))
        row_p = ec(tc.tile_pool(name="row", bufs=2))
        cols_p = ec(tc.tile_pool(name="cols", bufs=1))
        pts_p = ec(tc.tile_pool(name="pts", bufs=4))
        ot_p = ec(tc.tile_pool(name="ots", bufs=2))
        psP = ec(tc.tile_pool(name="psP", bufs=6, space="PSUM"))
        psS = ec(tc.tile_pool(name="psS", bufs=2, space="PSUM"))

        # ---- input DMAs (first-needed first) ------------------------------
        xt_s = [big_p.tile([128, ND, 512], BF16, tag=f"xt{h}", name=f"xt{h}")
                for h in range(2)]
        ws0 = ws_p.tile([128, ND, 128], BF16, tag="wqk", name="ws0")
        nc.sync.dma_start(ws0[:], wq_d[0])
        nc.sync.dma_start(xt_s[0][:, 0:4], xt_d[0][:, 0:4])
        nc.sync.dma_start(xt_s[0][:, 4:8], xt_d[0][:, 4:8])
        nc.sync.dma_start(xt_s[1][:, 0:4], xt_d[1][:, 0:4])
        nc.sync.dma_start(xt_s[1][:, 4:8], xt_d[1][:, 4:8])

        bm_s = const_p.tile([128, 256], BF16, tag="bm")
        nc.sync.dma_start(bm_s[:], bm_d[:])
        onc_s = const_p.tile([128, 1], BF16, tag="onc")
        nc.sync.dma_start(onc_s[:], onc_d[:])
        or32_s = const_p.tile([1, 128], F32, tag="or32")
        nc.sync.dma_start(or32_s[:].bitcast(F32R), or32_d[:].bitcast(F32R))
        orb_s = const_p.tile([1, 128], BF16, tag="orb")
        nc.sync.dma_start(orb_s[:], orb_d[:])
        o11_s = const_p.tile([1, 1], F32, tag="o11")
        nc.sync.dma_start(o11_s[:], o11_d[:])
        gc_s = const_p.tile([128, NE], F32, tag="gc")
        nc.sync.dma_start(gc_s[:], gc_d[:])
        bc_s = const_p.tile([128, NE], F32, tag="bc")
        nc.sync.dma_start(bc_s[:], bc_d[:])
        gr_s = const_p.tile([1, D], BF16, tag="gr")
        nc.sync.dma_start(gr_s[:], gr_d[:])
        br_s = const_p.tile([1, D], BF16, tag="br")
        nc.sync.dma_start(br_s[:], br_d[:])
        eps_c = const_p.tile([128, 1], F32, tag="eps")
        nc.gpsimd.memset(eps_c[:], LN_EPS)
        deps_c = const_p.tile([128, 1], F32, tag="deps")
        nc.gpsimd.memset(deps_c[:], float(D) * LN_EPS)
        # touch Exp once early so the ACT table load is off the critical path
        expwarm = const_p.tile([128, 1], F32, tag="expw")
        nc.scalar.activation(expwarm[:], eps_c[:], AFT.Exp)


        qc_s = big_p.tile([128, NE, TOK], BF16, tag="qc", name="qc")
        kc_s = big_p.tile([128, NE, TOK], BF16, tag="kc", name="kc")
        vc_s = big_p.tile([128, NTT, D], BF16, tag="vc", name="vc")
        yb_s = [big_p.tile([128, NE, 512], BF16, tag=f"yb{hh}", name=f"yb{hh}")
                for hh in range(2)]
        wv_s = big_p.tile([128, ND, D], BF16, tag="wv", name="wv")
        wo_s = big_p.tile([128, NE, D], BF16, tag="wo", name="wo")

        # gamma broadcast row [128, D] (general path only)
        gb_s = None
        if not gamma_trivial:
            gb_s = big_p.tile([128, D], BF16, tag="gb", name="gb")
            for fh in range(2):
                pg = psP.tile([128, 512], F32, tag="p512", name=f"pg{fh}")
                nc.tensor.matmul(pg[:], orb_s[:],
                                 gr_s[:, fh * 512:(fh + 1) * 512],
                                 start=True, stop=True)
                nc.scalar.activation(gb_s[:, fh * 512:(fh + 1) * 512], pg[:],
                                     AFT.Copy)

        # =========== q/k projections + fp32 sum-of-squares =================
        # sq staging: squares of the evicted (bf16) projections, reduced over
        # features at phase end by back-to-back ones-matmuls into fp32 PSUM.
        sqs_one = big_p.tile([128, 2, NE, 512], BF16, tag="sqs",
                             name="sqs")
        sqs = {"q": sqs_one, "k": sqs_one}  # WAR-reused across q->k
        ssq_rows = {}
        for w_d, dst, key in ((wq_d, qc_s, "q"), (wk_d, kc_s, "k")):
            for et in range(NE):
                if key == "q" and et == 0:
                    ws = ws0
                else:
                    ws = ws_p.tile([128, ND, 128], BF16, tag="wqk")
                    nc.sync.dma_start(ws[:], w_d[et])
                for h in range(2):
                    pp = psP.tile([128, 512], F32, tag="p512")
                    for dt in range(ND):
                        nc.tensor.matmul(
                            pp[:], ws[:, dt, :], xt_s[h][:, dt, :],
                            start=(dt == 0), stop=(dt == ND - 1),
                        )
                    dsl = dst[:, et, h * 512:(h + 1) * 512]
                    if (et + h) % 2 == 0:
                        nc.vector.tensor_scalar_mul(dsl, pp[:],
                                                    gc_s[:, et:et + 1])
                    else:
                        nc.scalar.activation(dsl, pp[:], AFT.Copy,
                                             scale=gc_s[:, et:et + 1])
                    nc.scalar.activation(sqs[key][:, h, et, :], pp[:],
                                         AFT.Square)
            ssq = [psS.tile([1, 512], F32, tag="stat", name=f"ssq_{key}{h}")
                   for h in range(2)]  # ring shared with pcol/l rows
            for h in range(2):
                for et in range(NE):
                    nc.tensor.matmul(
                        ssq[h][:], onc_s[:], sqs[key][:, h, et, :],
                        start=(et == 0), stop=(et == NE - 1),
                    )
            ssq_rows[key] = ssq

        # rqb = 1/(32*sqrt(ssq/D+eps)) = 1/sqrt(ssq+D*eps), broadcast
        # to [128, TOK] bf16; folded into qcn early so the attention phase
        # needs no score-PSUM RMW at all
        rqb_s = big_p.tile([128, TOK], BF16, tag="rqb", name="rqb")
        for h in range(2):
            sq_r = row_p.tile([1, 512], F32, tag="ra", bufs=2)
            nc.scalar.activation(sq_r[:], ssq_rows["q"][h][:], AFT.Sqrt,
                                 bias=deps_c[:1, :])
            rq_r = row_p.tile([1, 512], F32, tag="rb", bufs=2)
            nc.vector.reciprocal_approx_fast(rq_r[:], sq_r[:])
            rqr = row_p.tile([1, 512], F32, tag="rr", bufs=2)
            nc.scalar.activation(rqr[:].bitcast(F32R), rq_r[:], AFT.Copy)
            pb = psP.tile([128, 512], F32, tag="p512", name=f"rqb{h}")
            nc.tensor.matmul(pb[:], or32_s[:].bitcast(F32R),
                             rqr[:].bitcast(F32R), start=True, stop=True)
            nc.scalar.activation(rqb_s[:, h * 512:(h + 1) * 512],
                                 pb[:], AFT.Copy)
        qcn_s = big_p.tile([128, NE, TOK], BF16, tag="qcn", name="qcn")
        for et in range(NE):
            nc.vector.scalar_tensor_tensor(
                qcn_s[:, et, :], qc_s[:, et, :], 1.0, rqb_s[:],
                op0=ALU.bypass, op1=ALU.mult)

        # k divisor rows: sk = sqrt(ssq/D+eps), columnized to [128, 2*PPC]
        sk_rows = []
        for h in range(2):
            skr = row_p.tile([1, 512], F32, tag="rc", bufs=2)
            nc.scalar.activation(skr[:], ssq_rows["k"][h][:], AFT.Sqrt,
                                 bias=eps_c[:1, :], scale=1.0 / D)
            sk_rows.append(skr)
        pcol = psS.tile([128, 2 * PPC], F32, tag="stat", name="pcol_k")
        for i in range(2 * PPC):
            h, j = divmod(i, PPC)
            nc.tensor.matmul(pcol[:, i:i + 1],
                             sk_rows[h][:, j * 128:(j + 1) * 128],
                             o11_s[:], start=True, stop=True)
        sk_col = cols_p.tile([128, 2 * PPC], F32, tag="rkc")
        nc.scalar.activation(sk_col[:], pcol[:], AFT.Copy)
        rk_col = cols_p.tile([128, 2 * PPC], F32, tag="rkc2")
        nc.vector.reciprocal_approx_fast(rk_col[:], sk_col[:])

        # beta general path: fully normalize q and k into new buffers
        if beta_nonzero:
            bc32 = cols_p.tile([128, NE], F32, tag="bc32")
            nc.scalar.activation(bc32[:], bc_s[:], AFT.Identity,
                                 scale=1.0 / 32.0)
            qn_s = big_p.tile([128, NE, TOK], BF16, tag="qn", name="qn")
            kn_s = big_p.tile([128, NE, TOK], BF16, tag="kn", name="kn")
            rkb_s = big_p.tile([128, TOK], F32, tag="rkb", name="rkb")
            for h in range(2):
                rk_r = row_p.tile([1, 512], F32, tag="rb", bufs=2)
                nc.vector.reciprocal_approx_fast(rk_r[:], sk_rows[h][:])
                skr2 = row_p.tile([1, 512], F32, tag="rr", bufs=2)
                nc.scalar.activation(skr2[:].bitcast(F32R), rk_r[:],
                                     AFT.Copy)
                pb = psP.tile([128, 512], F32, tag="p512", name=f"rkb{h}")
                nc.tensor.matmul(pb[:], or32_s[:].bitcast(F32R),
                                 skr2[:].bitcast(F32R),
                                 start=True, stop=True)
                nc.scalar.activation(rkb_s[:, h * 512:(h + 1) * 512], pb[:],
                                     AFT.Copy)
            for et in range(NE):
                # qn = qc/sqb + beta/32 ; kn = kc/skb + beta
                t0 = sq_p.tile([128, TOK], BF16, tag="bq", bufs=2)
                nc.vector.scalar_tensor_tensor(
                    t0[:], qc_s[:, et, :], 1.0, rqb_s[:],
                    op0=ALU.bypass, op1=ALU.mult)
                nc.vector.tensor_scalar_add(
                    qn_s[:, et, :], t0[:], bc32[:, et:et + 1])
                t1 = sq_p.tile([128, TOK], BF16, tag="bk", bufs=2)
                nc.vector.scalar_tensor_tensor(
                    t1[:], kc_s[:, et, :], 1.0, rkb_s[:],
                    op0=ALU.bypass, op1=ALU.mult)
                nc.vector.tensor_scalar_add(
                    kn_s[:, et, :], t1[:], bc_s[:, et:et + 1])
            qc_use, kc_use = qn_s, kn_s
        else:
            qc_use, kc_use = qcn_s, kc_s

        # =========== v projection (token-major) + rstd_v ===================
        nc.sync.dma_start(wv_s[:], wv_d[:])
        # rstd_v is folded into the v eviction (exact: per-token partition
        # scale), so the softmax denominator stays rv-free.
        rv_col = cols_p.tile([128, NTT], F32, tag="rvc")
        sva_t = [cols_p.tile([128, 2], F32, tag="sva", bufs=NTT,
                             name=f"sva{tt}") for tt in range(NTT)]
        sv_col = cols_p.tile([128, NTT], F32, tag="svc")
        for tp in range(NTT // 2):
            for tt in (2 * tp, 2 * tp + 1):
                pv = [psP.tile([128, 512], F32, tag="p512",
                               name=f"pv{tt}_{fh}") for fh in range(2)]
                h, j = divmod(tt, 4)
                for dt in range(ND):
                    stat = xt_s[h][:, dt, j * 128:(j + 1) * 128]
                    for fh in range(2):
                        nc.tensor.matmul(
                            pv[fh][:], stat,
                            wv_s[:, dt, fh * 512:(fh + 1) * 512],
                            start=(dt == 0), stop=(dt == ND - 1),
                        )
                for fh in range(2):
                    sqv = sq_p.tile([128, 512], BF16, tag="sq")
                    nc.scalar.activation(sqv[:], pv[fh][:], AFT.Square,
                                         accum_out=sva_t[tt][:, fh:fh + 1])
                    # plain eviction: no rv dependency, fast PSUM turnover
                    dsl = vc_s[:, tt, fh * 512:(fh + 1) * 512]
                    if not gamma_trivial:
                        nc.vector.scalar_tensor_tensor(
                            dsl, pv[fh][:], 1.0,
                            gb_s[:, fh * 512:(fh + 1) * 512],
                            op0=ALU.bypass, op1=ALU.mult)
                    elif fh == 0:
                        nc.vector.tensor_copy(dsl, pv[fh][:])
                    else:
                        nc.scalar.activation(dsl, pv[fh][:], AFT.Copy)
                nc.vector.reduce_sum(sv_col[:, tt:tt + 1], sva_t[tt][:],
                                     axis=AX.X)
        # one batched rv chain (single Sqrt table use, one approx recip)
        svq = cols_p.tile([128, NTT], F32, tag="svq")
        nc.scalar.activation(svq[:], sv_col[:], AFT.Sqrt,
                             bias=eps_c[:], scale=1.0 / D)
        nc.vector.reciprocal_approx_fast(rv_col[:], svq[:])

        nc.sync.dma_start(wo_s[:], wo_d[:])

        # beta general path: v = v*rv + beta (rv folded here; mask op
        # then skips it; exact because unnormalized rows divide by l later
        # and beta*sum(p)/l = beta)
        if beta_nonzero:
            vn_s = big_p.tile([128, NTT, D], BF16, tag="vn", name="vn")
            bb_s = big_p.tile([128, D], BF16, tag="bb", name="bb")
            for fh in range(2):
                pbb = psP.tile([128, 512], F32, tag="p512", name=f"pbb{fh}")
                nc.tensor.matmul(pbb[:], orb_s[:],
                                 br_s[:, fh * 512:(fh + 1) * 512],
                                 start=True, stop=True)
                nc.scalar.activation(bb_s[:, fh * 512:(fh + 1) * 512],
                                     pbb[:], AFT.Copy)
            for tt in range(NTT):
                t2 = sq_p.tile([128, D], BF16, tag="bv", bufs=2)
                nc.vector.tensor_scalar_mul(t2[:], vc_s[:, tt, :],
                                            rv_col[:, tt:tt + 1])
                nc.vector.scalar_tensor_tensor(
                    vn_s[:, tt, :], t2[:], 1.0, bb_s[:],
                    op0=ALU.bypass, op1=ALU.add)
            vc_use = vn_s
        else:
            vc_use = vc_s

        # =========== attention (transposed scores) + output ================
        l_store = {}
        pts_store = {}
        pm_store = {}
        rlb_store = {}

        def attn_scores(sg):
            c0 = sg * 256
            st0 = psP.tile([128, 512], F32, tag="p512", name=f"st0_{sg}")
            st1 = psP.tile([128, 512], F32, tag="p512", name=f"st1_{sg}")
            for et in range(NE):
                nc.tensor.matmul(
                    st0[:, 0:256], kc_use[:, et, c0:c0 + 128],
                    qc_use[:, et, c0:c0 + 256],
                    start=(et == 0), stop=(et == NE - 1))
            for et in range(NE):
                nc.tensor.matmul(
                    st1[:, 0:128], kc_use[:, et, c0 + 128:c0 + 256],
                    qc_use[:, et, c0 + 128:c0 + 256],
                    start=(et == 0), stop=(et == NE - 1))
            if not beta_nonzero:
                sc0 = rk_col[:, 2 * sg:2 * sg + 1]
                sc1 = rk_col[:, 2 * sg + 1:2 * sg + 2]
            else:
                sc0 = sc1 = 1.0
            p0 = p01_p.tile([128, 256], BF16, tag="p0")
            nc.scalar.activation(p0[:], st0[:, 0:256], AFT.Exp, scale=sc0)
            p1 = p01_p.tile([128, 128], BF16, tag="p1")
            nc.scalar.activation(p1[:], st1[:, 0:128], AFT.Exp, scale=sc1)
            # mask-only tiles feed l; mask*rv tiles feed AV
            pm0 = pts_p.tile([128, 256], BF16, tag="pm0", name=f"pm0_{sg}")
            nc.vector.scalar_tensor_tensor(
                pm0[:], p0[:], 1.0, bm_s[:], op0=ALU.bypass, op1=ALU.mult)
            pm1 = pts_p.tile([128, 128], BF16, tag="pm1", name=f"pm1_{sg}")
            nc.vector.scalar_tensor_tensor(
                pm1[:], p1[:], 1.0, bm_s[:, 0:128],
                op0=ALU.bypass, op1=ALU.mult)
            if beta_nonzero:
                pts0, pts1 = pm0, pm1      # rv folded into vn instead
            else:
                pts0 = pts_p.tile([128, 256], BF16, tag="pts0",
                                  name=f"pts0_{sg}")
                nc.vector.scalar_tensor_tensor(
                    pts0[:], p0[:], rv_col[:, 2 * sg:2 * sg + 1], bm_s[:],
                    op0=ALU.mult, op1=ALU.mult)
                pts1 = pts_p.tile([128, 128], BF16, tag="pts1",
                                  name=f"pts1_{sg}")
                nc.vector.scalar_tensor_tensor(
                    pts1[:], p1[:], rv_col[:, 2 * sg + 1:2 * sg + 2],
                    bm_s[:, 0:128], op0=ALU.mult, op1=ALU.mult)
            pts_store[sg] = (pts0, pts1)
            pm_store[sg] = (pm0, pm1)
            # unnormalized row sums (per q) in fp32 psum, inverted
            # immediately (per-sg [1,128] reciprocals stay off the tail)
            l0 = psS.tile([1, 256], F32, tag="stat", name=f"l0_{sg}")
            nc.tensor.matmul(l0[:], onc_s[:], pm0[:], start=True, stop=True)
            l1 = psS.tile([1, 128], F32, tag="stat", name=f"l1_{sg}")
            nc.tensor.matmul(l1[:], onc_s[:], pm1[:], start=True, stop=True)
            l1r = row_p.tile([1, 128], F32, tag="l1r", bufs=2)
            nc.scalar.activation(l1r[:], l1[:], AFT.Copy)
            lsum = row_p.tile([1, 128], F32, tag="ls", bufs=2)
            nc.vector.tensor_tensor(lsum[:], l0[:, 128:256], l1r[:],
                                    op=ALU.add)
            rlsg = row_p.tile([1, 256], F32, tag=f"rl{sg}", bufs=1)
            nc.vector.reciprocal_approx_fast(rlsg[:, 0:128], l0[:, 0:128])
            nc.vector.reciprocal_approx_fast(rlsg[:, 128:256], lsum[:])
            l_store[sg] = rlsg

        def attn_av(sg):
            hh = sg // 2
            y0 = (sg - 2 * hh) * 256
            pts0, pts1 = pts_store[sg]
            for et in range(NE):
                yp = psP.tile([128, 256], F32, tag="p512",
                              name=f"yp{sg}_{et}")
                nc.tensor.matmul(
                    yp[:], vc_use[:, 2 * sg, et * 128:(et + 1) * 128],
                    pts0[:], start=True, stop=False, skip_group_check=True)
                nc.tensor.matmul(
                    yp[:, 128:256],
                    vc_use[:, 2 * sg + 1, et * 128:(et + 1) * 128],
                    pts1[:], start=False, stop=True, skip_group_check=True)
                dst = yb_s[hh][:, et, y0:y0 + 256]
                if et % 2 == 0:
                    nc.vector.tensor_copy(dst, yp[:])
                else:
                    nc.scalar.activation(dst, yp[:], AFT.Copy)

        def out_half(hh):
            # rl row [1, 512] fp32 for this half, broadcast to rlb [128, 512]
            lrow = row_p.tile([1, 512], F32, tag="lr", bufs=2)
            for sg in (2 * hh, 2 * hh + 1):
                j0 = (sg - 2 * hh) * 256
                nc.scalar.activation(
                    lrow[:, j0:j0 + 256].bitcast(F32R), l_store[sg][:],
                    AFT.Copy)
            lb = big_p.tile([128, 512], F32, tag=f"rlb{hh}", name=f"lb{hh}")
            rlb_store[hh] = lb
            pb = psP.tile([128, 512], F32, tag="p512", name=f"rlbp{hh}")
            nc.tensor.matmul(pb[:], or32_s[:].bitcast(F32R),
                             lrow[:].bitcast(F32R), start=True, stop=True)
            nc.scalar.activation(lb[:], pb[:], AFT.Copy)
            for ft in range(NE):
                po = psP.tile([128, 512], F32, tag="p512",
                              name=f"po{hh}_{ft}")
                for et in range(NE):
                    nc.tensor.matmul(
                        po[:], wo_s[:, et, ft * 128:(ft + 1) * 128],
                        yb_s[hh][:, et, :],
                        start=(et == 0), stop=(et == NE - 1))
                ots = ot_p.tile([128, 512], F32, tag="ot")
                nc.vector.scalar_tensor_tensor(
                    ots[:], po[:], 1.0, lb[:], op0=ALU.bypass,
                    op1=ALU.mult)
                nc.sync.dma_start(
                    ot_d[:, ft, 2 * hh:2 * hh + 2, :],
                    ots[:].rearrange("p (a b) -> p a b", a=2))

        attn_scores(0)
        attn_scores(1)
        attn_av(0)
        attn_scores(2)
        attn_av(1)
        attn_scores(3)
        out_half(0)
        attn_av(2)
        attn_av(3)
        out_half(1)

        if dbg:
            nc.sync.dma_start(qc_dbg[:], qc_use[:])
            nc.sync.dma_start(kc_dbg[:], kc_use[:])
            nc.sync.dma_start(vc_dbg[:], vc_use[:])
            nc.sync.dma_start(rqb_dbg[:], rqb_s[:])
            nc.sync.dma_start(rk_dbg[:], rk_col[:])
            nc.sync.dma_start(rv_dbg[:], rv_col[:])
            for sg in range(PPC):
                p0, p1 = pts_store[sg]
                nc.sync.dma_start(pts0_dbg[:, sg, :], p0[:])
                nc.sync.dma_start(pts1_dbg[:, sg, :], p1[:])
            for hh in range(2):
                nc.sync.dma_start(rl_dbg[hh], rlb_store[hh][:])
                nc.sync.dma_start(yb_dbg[hh], yb_s[hh][:])


def _rstd_rows(nc, row_p, ssq, eps_c, with32):
    """fp32 rstd rows from ssq psum rows: 1/sqrt(ssq/D+eps) (/32 if with32)."""
    rows = []
    for h in range(2):
        t = row_p.tile([1, 512], F32, tag="ra", bufs=2)
        nc.scalar.activation(t[:], ssq[h][:], AFT.Identity,
                             bias=eps_c[:1, :], scale=1.0 / D)
        s = row_p.tile([1, 512], F32, tag="rb", bufs=2)
        # sqrt(c*t): c=D gives 32*sqrt(t) (folds the 1/sqrt(D) score scale)
        nc.scalar.activation(s[:], t[:], AFT.Sqrt, bias=0.0,
                             scale=float(D) if with32 else 1.0)
        r = row_p.tile([1, 512], F32, tag="rc", bufs=2)
        nc.vector.reciprocal(r[:], s[:])
        rows.append(r)
    return rows


def _host_prep(x, Wq, Wk, Wv, Wo, gamma, beta):
    """Build per-core input dicts (numpy only, bf16 data tensors)."""
    x = np.ascontiguousarray(np.asarray(x, dtype=np.float32))
    Wq = np.asarray(Wq, dtype=np.float32)
    Wk = np.asarray(Wk, dtype=np.float32)
    Wv = np.asarray(Wv, dtype=np.float32)
    Wo = np.asarray(Wo, dtype=np.float32)
    gamma = np.asarray(gamma, dtype=np.float32)
    beta = np.asarray(beta, dtype=np.float32)

    xs = x.reshape(B, NSEG, SEG, D)[:, :, ::DIL, :].reshape(PAIRS, S, D)

    def center_T(W):
        Wc = W - W.mean(axis=0, keepdims=True)
        return np.ascontiguousarray(Wc.T)            # [d, e]

    WqT = center_T(Wq)
    WkT = center_T(Wk)
    WvT = center_T(Wv)
    WoT = np.ascontiguousarray(Wo.T)                 # [e, f]

    wq_h = np.ascontiguousarray(
        WqT.reshape(ND, 128, NE, 128).transpose(2, 1, 0, 3)).astype(NPBF)
    wk_h = np.ascontiguousarray(
        WkT.reshape(ND, 128, NE, 128).transpose(2, 1, 0, 3)).astype(NPBF)
    wv_h = np.ascontiguousarray(
        WvT.reshape(ND, 128, D).transpose(1, 0, 2)).astype(NPBF)
    wo_h = np.ascontiguousarray(
        WoT.reshape(NE, 128, D).transpose(1, 0, 2)).astype(NPBF)

    r = np.arange(128)[:, None]
    c = np.arange(256)[None, :]
    bm = (c >= r).astype(NPBF)                       # maskT[k, q]

    onc = np.ones((128, 1), dtype=NPBF)
    orb = np.ones((1, 128), dtype=NPBF)
    or32 = np.ones((1, 128), dtype=np.float32)
    o11 = np.ones((1, 1), dtype=np.float32)
    gcol = np.ascontiguousarray(gamma.reshape(NE, 128).T).astype(np.float32)
    bcol = np.ascontiguousarray(beta.reshape(NE, 128).T).astype(np.float32)
    grow = gamma.reshape(1, D).astype(NPBF)
    brow = beta.reshape(1, D).astype(NPBF)

    shared = {
        "wq": wq_h, "wk": wk_h, "wv": wv_h, "wo": wo_h,
        "bm": bm, "onc": onc, "or32": or32, "orb": orb, "o11": o11,
        "gc": gcol, "gr": grow, "bc": bcol, "br": brow,
    }

    in_maps = []
    for cidx in range(NCORES):
        toks = xs[cidx * PPC:(cidx + 1) * PPC].reshape(TOK, D)
        xT = toks.T.astype(NPBF)                     # [d, t] bf16
        xt_h = np.ascontiguousarray(
            xT.reshape(ND, 128, 2, 512).transpose(2, 1, 0, 3))  # [h,p,dt,t]
        m = dict(shared)
        m["xt"] = xt_h
        in_maps.append(m)
    gamma_trivial = bool(np.all(gamma == 1.0))
    beta_nonzero = bool(np.any(beta != 0.0))
    return in_maps, gamma_trivial, beta_nonzero


def _get_module(gamma_trivial: bool, beta_nonzero: bool, dbg: bool = False):
    key = ("mod", gamma_trivial, beta_nonzero, dbg)
    if key not in _CACHE:
        _CACHE[key] = _build_module(gamma_trivial, beta_nonzero, dbg)
    return _CACHE[key]


def _assemble(results):
    """[core]["ot"]: [128, NE, PPC, S] -> full [B, N, D] output."""
    out = np.zeros((PAIRS, SEG, D), dtype=np.float32)
    for cidx in range(NCORES):
        ot = results[cidx]["ot"]                     # [p, ft, sg, j]
        arr = ot.transpose(2, 3, 1, 0).reshape(PPC, S, D)
        out[cidx * PPC:(cidx + 1) * PPC, ::DIL, :] = arr
    return out.reshape(B, N, D)


def kernel(x, Wq, Wk, Wv, Wo, gamma, beta):
    in_maps, gamma_trivial, beta_nonzero = _host_prep(
        x, Wq, Wk, Wv, Wo, gamma, beta)
    nc = _get_module(gamma_trivial, beta_nonzero)
    res = run_bass_kernel_spmd(nc, in_maps, core_ids=list(range(NCORES)))
    return _assemble(res.results)
